# revision 18
# baseline (speedup 1.0000x reference)
"""Differentiable palette quantization on 8 Trainium2 NeuronCores.

Math: for each image b, pixel x, palette p_k (k=64):
    w = softmax_k(-|x - p_k|^2 / T);  out = sum_k w_k p_k
Softmax is invariant to the per-pixel |x|^2 term, so the logit reduces to
    scale*dot(x, p_k) + bias_k,  scale = 2/T, bias_k = -|p_k|^2/T.

Sharding: pure data parallel, 2 images per core, stacked on partitions
(64+64 palette entries) sharing the pixel stream via a block-diagonal
stationary matrix.  The dot matmul runs in bf16 with an exact hi/lo
fixup folded into the contraction dim.

v3 design (vs the 82us ACT-bound baseline):
 1. The exp is SPLIT between ACT and DVE.  Even rounds run the real exp
    on ACT (fp16 out).  Odd rounds run on DVE as an exp2 bit-trick: the
    int16 value round(1024*log2(e)*logit + 15360) IS the fp16 bit
    pattern of 2^y ~ e^logit (Schraudolph).  Softmax cancels the
    systematic part; measured end-to-end error ~7e-3 vs the 2e-2 gate.
 2. Both the scale and the bias of each engine's exp input are folded
    into the mm1 contraction: two per-engine stationary palettes
    (scale*p for ACT, 1024*log2e*scale*p for DVE) carry two extra hi/lo
    bias rows against constant-1.0 pixel rows.  ACT runs exp with
    scale=1/bias=0; DVE is a single convert-to-int16 tensor_scalar.
 3. The PE queue issues round r+1's three mm1 matmuls BEFORE round r's
    twelve weighted-sum matmuls, so the ACT exp of round r and the DVE
    exp of round r+1 genuinely overlap (the in-order PE queue otherwise
    serializes the two engines).
 4. No divide epilogue: psum2 blocks (numerators + softmax denominators)
    are copied PSUM->SBUF as fp16 by whichever of ACT/DVE the round
    parity frees, DMA'd raw to DRAM, and the host divides.
 5. DMA queues: input pixel chunks on sync HWDGE, palettes/constants on
    gpsimd SWDGE, output blocks on gpsimd with the last two on sync -
    the Scalar and Vector queues carry no DMA.
"""

import os
import sys

for _p in ("/opt/trn_rl_repo", os.path.expanduser("~/.axon_site/_ro/trn_rl_repo")):
    if os.path.isdir(_p) and _p not in sys.path:
        sys.path.insert(0, _p)

import numpy as np

import concourse.bass as bass
import concourse.tile as tile
from concourse import bacc, mybir
from concourse.bass_utils import run_bass_kernel_spmd

# problem constants (hardcoded per contract)
B, H, W, C, K = 16, 256, 256, 3, 64
NCORES = 8
IMGS_PER_CORE = B // NCORES            # 2
P = H * W                              # 65536 pixel-pairs per core
NQ = 4                                 # PE row-tile quarters
QP = P // NQ                           # 16384 pixels per quarter
RN = 512                               # pixels per strip-task
ROUNDS = QP // RN                      # 32 chunks per strip
NTASKS = NQ * ROUNDS                   # 128 matmul tasks
SPT = 60                               # weighted-sum subtiles per psum2 block
NSUB = NTASKS * 4                      # 512 subtiles of 128 pixels
NBLK = (NSUB + SPT - 1) // SPT         # 9 output blocks
KR = 20                                # contraction rows: 3x6 hi/lo + 2 bias

LOG2E = float(np.log2(np.e))

# tuning knobs (env-overridable for experiments)
DVE_PAT = os.environ.get("PALQ_DVE_PAT", "AD")  # cycle of A/D per round
OUT_DT = os.environ.get("PALQ_OUT_DT", "float16")  # psum2 copy-out dtype


def build_bass(scale: float):
    nc = bacc.Bacc("TRN2", target_bir_lowering=False, debug=False)
    f32 = mybir.dt.float32
    f16 = mybir.dt.float16
    bf16 = mybir.dt.bfloat16
    out_dt = getattr(mybir.dt, OUT_DT)
    i16 = mybir.dt.int16

    xin = nc.dram_tensor("xin", [NQ, 18, QP], bf16, kind="ExternalInput")
    ones = nc.dram_tensor("ones", [2, QP], bf16, kind="ExternalInput")
    palta = nc.dram_tensor("palta", [128, 128], bf16, kind="ExternalInput")
    paltd = nc.dram_tensor("paltd", [128, 128], bf16, kind="ExternalInput")
    palw_hi = nc.dram_tensor("palw_hi", [128, 8], f16, kind="ExternalInput")
    out = nc.dram_tensor("out", [NBLK, 128, 8 * SPT], out_dt,
                         kind="ExternalOutput")

    with tile.TileContext(nc) as tc:
        import contextlib
        with contextlib.ExitStack() as ctx:
            singles = ctx.enter_context(tc.tile_pool(name="singles", bufs=1))
            epool = ctx.enter_context(tc.tile_pool(name="epool", bufs=4))
            # one single-bank psum1 pool PER TASK SLOT (each double
            # buffered): every exp is a 512-col op whose psum WAR loop
            # (exp -> next-next round's mm1 -> exp) is ~1.5us, well under
            # the 2-round spacing.  3x2 + 2 = 8 PSUM banks exactly.
            ps1 = [ctx.enter_context(
                tc.tile_pool(name=f"ps1{m}", bufs=2, space="PSUM"))
                for m in range(3)]
            ps2 = ctx.enter_context(tc.tile_pool(name="ps2", bufs=2, space="PSUM"))
            opool = ctx.enter_context(tc.tile_pool(name="opool", bufs=3))

            # resident input pixels: quarter j on partitions [32j, 32j+20);
            # rows 18,19 are the constant-1.0 bias rows
            xsb = singles.tile([128, QP], bf16)

            # round 0 is all-ACT on strips 0,1,2: palta + those chunks go
            # FIRST on sync.  paltd + strip-3 ride the gpsimd queue.
            palta_sb = singles.tile([128, 128], bf16)
            nc.sync.dma_start(out=palta_sb, in_=palta.ap())
            for j in range(3):
                nc.sync.dma_start(out=xsb[32 * j:32 * j + 18, 0:512],
                                  in_=xin.ap()[j, :, 0:512])
            paltd_sb = singles.tile([128, 128], bf16)
            nc.gpsimd.dma_start(out=paltd_sb, in_=paltd.ap())
            nc.gpsimd.dma_start(out=xsb[96:96 + 18, 0:512],
                                in_=xin.ap()[3, :, 0:512])
            for j in range(NQ):
                nc.gpsimd.dma_start(out=xsb[32 * j + 18:32 * j + 20, :],
                                    in_=ones.ap())
            palw_sb = singles.tile([128, 8], f16)
            nc.gpsimd.dma_start(out=palw_sb, in_=palw_hi.ap())

            # pre-warm the ACT exp table while input DMAs stream
            warm = singles.tile([1, 1], f32)
            nc.scalar.activation(out=warm,
                                 in_=nc.const_aps.scalar_like(0.0, warm),
                                 func=mybir.ActivationFunctionType.Exp)

            # remaining pixel chunks on sync; chunk-major issue order with
            # small first chunks so round 0's columns land early
            bounds = [0, 512, 1536, 2560, 4096, 6144, 8192, 10240, 12288,
                      14336, QP]
            for h in range(len(bounds) - 1):
                sl = slice(bounds[h], bounds[h + 1])
                for j in range(NQ):
                    if h == 0:
                        continue  # issued above
                    # strips 0,1 feed from sync; strips 2,3 from gpsimd so
                    # neither queue's descriptor-gen rate starves the PE
                    eng = nc.sync if j < 2 else nc.gpsimd
                    eng.dma_start(out=xsb[32 * j:32 * j + 18, sl],
                                  in_=xin.ap()[j, :, sl])

            # 128 matmul tasks i -> (strip j = i%4, chunk k = i//4), three
            # per 1536-col round (strips always distinct mod 4).  psum1 =
            # 2x3 banks, psum2 = 2x1 banks: exactly 8 PSUM banks.
            # Round r+1's mm1s are issued BEFORE round r's mm2s so the two
            # exp engines overlap; mm2s trail one round behind.
            NRND = (NTASKS + 2) // 3               # 43 (last has 2 tasks)
            pat = DVE_PAT

            state = {"psum2": None, "s": 0}

            def flush(e_sb, nt, use_dve):
                """12 weighted-sum matmuls for a finished e tile + block
                copies when psum2 fills.  The engine NOT running exps this
                round does the PSUM->SBUF copy."""
                s = state["s"]
                for t in range(4 * nt):
                    if state["psum2"] is None:
                        state["psum2"] = ps2.tile([128, 8 * SPT], f32,
                                                  name="psum2")
                    psum2 = state["psum2"]
                    u = s % SPT
                    nc.tensor.matmul(
                        out=psum2[:, 8 * u:8 * u + 8],
                        lhsT=e_sb[:, 128 * t:128 * (t + 1)],
                        rhs=palw_sb,
                        start=True, stop=True,
                    )
                    s += 1
                    if s % SPT == 0 or s == NSUB or s == NSUB - 16:
                        b = (s - 1) // SPT
                        v0 = 0 if s - SPT * b in (0, SPT) else (
                            (s - 1) % SPT + 1 - 16 if s == NSUB - 16
                            else NSUB - 16 - SPT * b)
                        v1 = (s - 1) % SPT + 1
                        nu = v1 - v0
                        ob = opool.tile([128, 8 * nu], out_dt, name="ob")
                        if use_dve:
                            nc.scalar.activation(
                                out=ob, in_=psum2[:, 8 * v0:8 * v1],
                                func=mybir.ActivationFunctionType.Copy)
                        else:
                            nc.vector.tensor_scalar_add(
                                out=ob, in0=psum2[:, 8 * v0:8 * v1],
                                scalar1=0.0)
                        oeng = nc.gpsimd if b < NBLK - 2 else nc.sync
                        oeng.dma_start(out=out.ap()[b, :, 8 * v0:8 * v1],
                                       in_=ob)
                        if s % SPT == 0 or s == NSUB:
                            state["psum2"] = None
                state["s"] = s

            # mm2s trail TWO rounds behind their exp: the in-order PE queue
            # then never stalls (each mm2 batch consumes an e tile finished
            # two rounds ago) and the two exp engines overlap fully
            # engine per TASK: a [A,A,D]/[D,D,A] two-round cycle gives ACT
            # 66 of the 128 tasks and DVE 62 (balancing their 612/658 ns
            # per-512-col-op costs) with BOTH engines active every round.
            # Round 0 is all-ACT so the start only waits on palta.
            def engines_of(r, nt):
                if r == 0:
                    return "AAA"[:nt]
                return ("AAD" if r % 2 == 0 else "DDA")[:nt]

            pending = []
            for r in range(NRND):
                tasks = [3 * r + m for m in range(3) if 3 * r + m < NTASKS]
                nt = len(tasks)
                engs = engines_of(r, nt)
                psum1 = [ps1[m].tile([128, RN], f32, name=f"psum1{m}")
                         for m in range(nt)]
                for m, i in enumerate(tasks):
                    j, k = i % NQ, i // NQ
                    psl = slice(32 * j, 32 * j + KR)
                    nc.tensor.matmul(
                        out=psum1[m][:, 0:RN],
                        lhsT=(paltd_sb if engs[m] == "D" else
                              palta_sb)[psl, :],
                        rhs=xsb[psl, RN * k:RN * (k + 1)],
                        start=True, stop=True,
                        tile_position=(32 * j, 0),
                    )
                if len(pending) == 2:
                    flush(*pending.pop(0))
                e_sb = epool.tile([128, 3 * RN], f16)
                for m in range(nt):
                    if engs[m] == "D":
                        nc.vector.tensor_scalar_add(
                            out=e_sb[:, RN * m:RN * (m + 1)].bitcast(i16),
                            in0=psum1[m][:, 0:RN],
                            scalar1=0.0,
                        )
                    else:
                        nc.scalar.activation(
                            out=e_sb[:, RN * m:RN * (m + 1)],
                            in_=psum1[m][:, 0:RN],
                            func=mybir.ActivationFunctionType.Exp,
                        )
                # the engine with the lighter duty this round owns any
                # psum2 block copy that the trailing flush produces (flush
                # of round r runs during round r+2, same parity): even
                # rounds DVE is lighter -> vector copy; odd -> scalar
                pending.append((e_sb, nt, r % 2 == 1))
            for p in pending:
                flush(*p)

    nc.compile()
    return nc


def _host_prep(images, palettes, scale):
    """Per-core input arrays. images [16,256,256,3] f32, palettes [16,64,3].
    scale = 2/temperature; the softmax logit is scale*dot + bias_k."""
    import ml_dtypes

    bf = ml_dtypes.bfloat16
    imgs = np.ascontiguousarray(images, np.float32).reshape(B, P, C)
    pals = np.ascontiguousarray(palettes, np.float32)
    s1 = 1024.0 * LOG2E * scale
    in_maps = []
    ones2 = np.ones((2, QP), bf)
    for core in range(NCORES):
        ia, ib = imgs[2 * core], imgs[2 * core + 1]
        # per-quarter channel rows: [rgbA | rgbB] on the contraction dim
        x6 = np.empty((NQ, 6, QP), np.float32)
        x6[:, 0:3] = ia.reshape(NQ, QP, C).transpose(0, 2, 1)
        x6[:, 3:6] = ib.reshape(NQ, QP, C).transpose(0, 2, 1)
        xh = x6.astype(bf)
        xl = (x6 - xh.astype(np.float32)).astype(bf)
        xin = np.concatenate([xh, xl, xh], axis=1)           # [NQ, 18, QP]

        pa, pb = pals[2 * core], pals[2 * core + 1]
        p6 = np.zeros((6, 128), np.float64)   # block-diag [pA^T | pB^T]
        p6[0:3, 0:64] = pa.T
        p6[3:6, 64:128] = pb.T
        # per-partition logit bias (-|p|^2 * scale/2, i.e. -|p|^2/T)
        eb = np.empty((128,), np.float64)
        eb[0:64] = -0.5 * scale * (pa.astype(np.float64) ** 2).sum(-1)
        eb[64:128] = -0.5 * scale * (pb.astype(np.float64) ** 2).sum(-1)

        def make_palt(mult, bias):
            """[KR=20,128] = [qh|qh|ql|bias_h|bias_l] for q = mult*p6."""
            q = mult * p6
            qh = q.astype(bf)
            ql = (q - qh.astype(np.float64)).astype(bf)
            bh = bias.astype(bf)
            bl = (bias - bh.astype(np.float64)).astype(bf)
            rows = np.concatenate(
                [qh, qh, ql, bh[None, :], bl[None, :]], axis=0)
            full = np.zeros((128, 128), bf)
            for j in range(NQ):
                full[32 * j:32 * j + KR] = rows
            return full

        # ACT rounds: psum = scale*dot + eb;  e = exp(psum)
        palta = make_palt(scale, eb)
        # DVE rounds: psum = s1*dot + (1024*log2e*eb + 15360.5);
        # int16(psum) is the fp16 bit pattern of 2^(log2e*logit)
        paltd = make_palt(s1, 1024.0 * LOG2E * eb + 15360.5)

        palw = np.zeros((128, 8), np.float32)
        palw[0:64, 0:3] = pa
        palw[0:64, 3] = 1.0
        palw[64:128, 4:7] = pb
        palw[64:128, 7] = 1.0

        in_maps.append({"xin": xin, "ones": ones2, "palta": palta,
                        "paltd": paltd, "palw_hi": palw.astype(np.float16)})
    return in_maps


def _host_post(results):
    """results[core]["out"] [9, 128, 480] (numer rgb + denom, 2 images
    interleaved on col%8) -> [16, 256, 256, 3]."""
    # subtile s -> (round r, t): s = 12r + t; task i = 3r + t//4 ->
    # (strip j = i%4, chunk k = i//4), q = t%4;
    # pixel = j*QP + k*512 + q*128 + row; stored at block s//SPT col s%SPT
    s_arr = np.arange(NSUB)
    r_arr = np.minimum(s_arr // 12, (NTASKS + 2) // 3 - 1)
    t_arr = s_arr - 12 * r_arr
    i_arr = 3 * r_arr + t_arr // 4
    base = (i_arr % NQ) * QP + (i_arr // NQ) * RN + (t_arr % 4) * 128
    out = np.empty((B, P, C), np.float32)
    for core in range(NCORES):
        o = np.asarray(results[core]["out"], np.float32)  # [9,128,480]
        o = o.reshape(NBLK, 128, SPT, 2, 4)               # [b,row,u,img,ch]
        q = o[..., 0:3] / o[..., 3:4]
        dec = np.empty((IMGS_PER_CORE, P, C), np.float32)
        for s in range(NSUB):
            b, u = s // SPT, s % SPT
            dec[0, base[s]:base[s] + 128, :] = q[b, :, u, 0]
            dec[1, base[s]:base[s] + 128, :] = q[b, :, u, 1]
        out[2 * core] = dec[0]
        out[2 * core + 1] = dec[1]
    return out.reshape(B, H, W, C)


_CACHE = {}


def _get_nc(scale: float):
    key = (round(float(scale), 12), DVE_PAT, OUT_DT)
    if key not in _CACHE:
        _CACHE[key] = build_bass(scale)
    return _CACHE[key]


def kernel(images, palettes, temperature, _trace=False):
    scale = 2.0 / float(np.asarray(temperature))
    nc = _get_nc(scale)
    in_maps = _host_prep(images, palettes, scale)
    res = run_bass_kernel_spmd(nc, in_maps, core_ids=list(range(NCORES)),
                               trace=_trace)
    out = _host_post(res.results)
    if _trace:
        kernel.last_result = res
    return out


# revision 20
# speedup vs baseline: 1.2457x; 1.2457x over previous
"""Differentiable palette quantization on 8 Trainium2 NeuronCores.

Math: for each image b, pixel x, palette p_k (k=64):
    w = softmax_k(-|x - p_k|^2 / T);  out = sum_k w_k p_k
Softmax is invariant to the per-pixel |x|^2 term, so the logit reduces to
    scale*dot(x, p_k) + bias_k,  scale = 2/T, bias_k = -|p_k|^2/T.

Sharding: pure data parallel, 2 images per core, stacked on partitions
(64+64 palette entries) sharing the pixel stream via a block-diagonal
stationary matrix.  The dot matmul runs in bf16 with an exact hi/lo
fixup folded into the contraction dim.

v3 design (vs the 82us ACT-bound baseline):
 1. The exp is SPLIT between ACT and DVE.  Even rounds run the real exp
    on ACT (fp16 out).  Odd rounds run on DVE as an exp2 bit-trick: the
    int16 value round(1024*log2(e)*logit + 15360) IS the fp16 bit
    pattern of 2^y ~ e^logit (Schraudolph).  Softmax cancels the
    systematic part; measured end-to-end error ~7e-3 vs the 2e-2 gate.
 2. Both the scale and the bias of each engine's exp input are folded
    into the mm1 contraction: two per-engine stationary palettes
    (scale*p for ACT, 1024*log2e*scale*p for DVE) carry two extra hi/lo
    bias rows against constant-1.0 pixel rows.  ACT runs exp with
    scale=1/bias=0; DVE is a single convert-to-int16 tensor_scalar.
 3. The PE queue issues round r+1's three mm1 matmuls BEFORE round r's
    twelve weighted-sum matmuls, so the ACT exp of round r and the DVE
    exp of round r+1 genuinely overlap (the in-order PE queue otherwise
    serializes the two engines).
 4. No divide epilogue: psum2 blocks (numerators + softmax denominators)
    are copied PSUM->SBUF as fp16 by whichever of ACT/DVE the round
    parity frees, DMA'd raw to DRAM, and the host divides.
 5. DMA queues: input pixel chunks on sync HWDGE, palettes/constants on
    gpsimd SWDGE, output blocks on gpsimd with the last two on sync -
    the Scalar and Vector queues carry no DMA.
"""

import os
import sys

for _p in ("/opt/trn_rl_repo", os.path.expanduser("~/.axon_site/_ro/trn_rl_repo")):
    if os.path.isdir(_p) and _p not in sys.path:
        sys.path.insert(0, _p)

import numpy as np

import concourse.bass as bass
import concourse.tile as tile
from concourse import bacc, mybir
from concourse.bass_utils import run_bass_kernel_spmd

# problem constants (hardcoded per contract)
B, H, W, C, K = 16, 256, 256, 3, 64
NCORES = 8
IMGS_PER_CORE = B // NCORES            # 2
P = H * W                              # 65536 pixel-pairs per core
NQ = 4                                 # PE row-tile quarters
QP = P // NQ                           # 16384 pixels per quarter
RN = 512                               # pixels per strip-task
ROUNDS = QP // RN                      # 32 chunks per strip
NTASKS = NQ * ROUNDS                   # 128 matmul tasks
SPT = 60                               # weighted-sum subtiles per psum2 block
NSUB = NTASKS * 4                      # 512 subtiles of 128 pixels
NBLK = (NSUB + SPT - 1) // SPT         # 9 output blocks
KR = 20                                # contraction rows: 3x6 hi/lo + 2 bias

LOG2E = float(np.log2(np.e))

# tuning knobs (env-overridable for experiments)
DVE_PAT = os.environ.get("PALQ_DVE_PAT", "AD")  # cycle of A/D per round
OUT_DT = os.environ.get("PALQ_OUT_DT", "float16")  # psum2 copy-out dtype


def build_bass(scale: float):
    nc = bacc.Bacc("TRN2", target_bir_lowering=False, debug=False)
    f32 = mybir.dt.float32
    f16 = mybir.dt.float16
    bf16 = mybir.dt.bfloat16
    out_dt = getattr(mybir.dt, OUT_DT)
    i16 = mybir.dt.int16

    xin = nc.dram_tensor("xin", [NQ, 18, QP], bf16, kind="ExternalInput")
    ones = nc.dram_tensor("ones", [2, QP], bf16, kind="ExternalInput")
    palta = nc.dram_tensor("palta", [128, 128], bf16, kind="ExternalInput")
    paltd = nc.dram_tensor("paltd", [128, 128], bf16, kind="ExternalInput")
    palw_hi = nc.dram_tensor("palw_hi", [128, 8], f16, kind="ExternalInput")
    out = nc.dram_tensor("out", [NBLK, 128, 8 * SPT], out_dt,
                         kind="ExternalOutput")

    with tile.TileContext(nc) as tc:
        import contextlib
        with contextlib.ExitStack() as ctx:
            singles = ctx.enter_context(tc.tile_pool(name="singles", bufs=1))
            epool = ctx.enter_context(tc.tile_pool(name="epool", bufs=4))
            # psum1 = two pools per round: a 2-bank [1024] tile (tasks 0,1)
            # and a 1-bank [512] tile (task 2), each double buffered.  The
            # round's exp runs as ONE op per engine (1024 on one, 512 on
            # the other, pair-alternating), so the exp makespan stays under
            # the PE's ~1850ns pair budget.  4+2+2 = 8 PSUM banks.
            psA = ctx.enter_context(tc.tile_pool(name="psA", bufs=2, space="PSUM"))
            psB = ctx.enter_context(tc.tile_pool(name="psB", bufs=2, space="PSUM"))
            ps2 = ctx.enter_context(tc.tile_pool(name="ps2", bufs=2, space="PSUM"))
            opool = ctx.enter_context(tc.tile_pool(name="opool", bufs=3))

            # resident input pixels: quarter j on partitions [32j, 32j+20);
            # rows 18,19 are the constant-1.0 bias rows
            xsb = singles.tile([128, QP], bf16)

            # round 0 is all-ACT on strips 0,1,2: palta + those chunks go
            # FIRST on sync.  paltd + strip-3 ride the gpsimd queue.
            palta_sb = singles.tile([128, 128], bf16)
            nc.sync.dma_start(out=palta_sb, in_=palta.ap())
            for j in range(3):
                nc.sync.dma_start(out=xsb[32 * j:32 * j + 18, 0:512],
                                  in_=xin.ap()[j, :, 0:512])
            paltd_sb = singles.tile([128, 128], bf16)
            nc.gpsimd.dma_start(out=paltd_sb, in_=paltd.ap())
            nc.gpsimd.dma_start(out=xsb[96:96 + 18, 0:512],
                                in_=xin.ap()[3, :, 0:512])
            for j in range(NQ):
                nc.gpsimd.dma_start(out=xsb[32 * j + 18:32 * j + 20, :],
                                    in_=ones.ap())
            palw_sb = singles.tile([128, 8], f16)
            nc.gpsimd.dma_start(out=palw_sb, in_=palw_hi.ap())

            # pre-warm the ACT exp table while input DMAs stream
            warm = singles.tile([1, 1], f32)
            nc.scalar.activation(out=warm,
                                 in_=nc.const_aps.scalar_like(0.0, warm),
                                 func=mybir.ActivationFunctionType.Exp)

            # remaining pixel chunks on sync; chunk-major issue order with
            # small first chunks so round 0's columns land early
            bounds = [0, 512, 1536, 2560, 4096, 6144, 8192, 10240, 12288,
                      14336, QP]
            for h in range(len(bounds) - 1):
                sl = slice(bounds[h], bounds[h + 1])
                for j in range(NQ):
                    if h == 0:
                        continue  # issued above
                    # strips 0,1 feed from sync; strips 2,3 from gpsimd so
                    # neither queue's descriptor-gen rate starves the PE
                    eng = nc.sync if j < 2 else nc.gpsimd
                    eng.dma_start(out=xsb[32 * j:32 * j + 18, sl],
                                  in_=xin.ap()[j, :, sl])

            # 128 matmul tasks i -> (strip j = i%4, chunk k = i//4), three
            # per 1536-col round (strips always distinct mod 4).  psum1 =
            # 2x3 banks, psum2 = 2x1 banks: exactly 8 PSUM banks.
            # Round r+1's mm1s are issued BEFORE round r's mm2s so the two
            # exp engines overlap; mm2s trail one round behind.
            NRND = (NTASKS + 2) // 3               # 43 (last has 2 tasks)
            pat = DVE_PAT

            state = {"psum2": None, "s": 0}

            def flush(e_sb, nt, use_dve):
                """12 weighted-sum matmuls for a finished e tile + block
                copies when psum2 fills.  The engine NOT running exps this
                round does the PSUM->SBUF copy."""
                s = state["s"]
                for t in range(4 * nt):
                    if state["psum2"] is None:
                        state["psum2"] = ps2.tile([128, 8 * SPT], f32,
                                                  name="psum2")
                    psum2 = state["psum2"]
                    u = s % SPT
                    nc.tensor.matmul(
                        out=psum2[:, 8 * u:8 * u + 8],
                        lhsT=e_sb[:, 128 * t:128 * (t + 1)],
                        rhs=palw_sb,
                        start=True, stop=True,
                    )
                    s += 1
                    if s % SPT == 0 or s == NSUB or s == NSUB - 16:
                        b = (s - 1) // SPT
                        v0 = 0 if s - SPT * b in (0, SPT) else (
                            (s - 1) % SPT + 1 - 16 if s == NSUB - 16
                            else NSUB - 16 - SPT * b)
                        v1 = (s - 1) % SPT + 1
                        nu = v1 - v0
                        ob = opool.tile([128, 8 * nu], out_dt, name="ob")
                        if use_dve:
                            nc.scalar.activation(
                                out=ob, in_=psum2[:, 8 * v0:8 * v1],
                                func=mybir.ActivationFunctionType.Copy)
                        else:
                            nc.vector.tensor_scalar_add(
                                out=ob, in0=psum2[:, 8 * v0:8 * v1],
                                scalar1=0.0)
                        oeng = nc.gpsimd if b < NBLK - 2 else nc.sync
                        oeng.dma_start(out=out.ap()[b, :, 8 * v0:8 * v1],
                                       in_=ob)
                        if s % SPT == 0 or s == NSUB:
                            state["psum2"] = None
                state["s"] = s

            # mm2s trail TWO rounds behind their exp: the in-order PE queue
            # then never stalls (each mm2 batch consumes an e tile finished
            # two rounds ago) and the two exp engines overlap fully
            # Engine split per round: tasks 0,1 (the 1024-col psA tile) on
            # one engine, task 2 (psB, 512 cols) on the other; which engine
            # gets the big slice alternates per round, balancing ACT's
            # 0.83ns/col against DVE's 1.04ns/col over round pairs.
            pending = []
            for r in range(NRND):
                tasks = [3 * r + m for m in range(3) if 3 * r + m < NTASKS]
                nt = len(tasks)
                big_on_act = r % 2 == 0
                psum1a = psA.tile([128, 2 * RN], f32, name="psum1a")
                psum1b = psB.tile([128, RN], f32, name="psum1b") \
                    if nt == 3 else None
                for m, i in enumerate(tasks):
                    j, k = i % NQ, i // NQ
                    psl = slice(32 * j, 32 * j + KR)
                    on_act = big_on_act if m < 2 else not big_on_act
                    nc.tensor.matmul(
                        out=psum1a[:, RN * m:RN * (m + 1)] if m < 2
                        else psum1b[:, 0:RN],
                        lhsT=(palta_sb if on_act else paltd_sb)[psl, :],
                        rhs=xsb[psl, RN * k:RN * (k + 1)],
                        start=True, stop=True,
                        tile_position=(32 * j, 0),
                    )
                if len(pending) == 2:
                    flush(*pending.pop(0))
                e_sb = epool.tile([128, 3 * RN], f16)
                segs = [(psum1a, 0, 2 * RN)]
                if nt == 3:
                    segs.append((psum1b, 2 * RN, 3 * RN))
                for si, (src, c0, c1) in enumerate(segs):
                    on_act = big_on_act if si == 0 else not big_on_act
                    if on_act:
                        nc.scalar.activation(
                            out=e_sb[:, c0:c1], in_=src[:, 0:c1 - c0],
                            func=mybir.ActivationFunctionType.Exp,
                        )
                    else:
                        nc.vector.tensor_scalar_add(
                            out=e_sb[:, c0:c1].bitcast(i16),
                            in0=src[:, 0:c1 - c0],
                            scalar1=0.0,
                        )
                # the engine holding only the 512-col slice this round owns
                # any psum2 block copy the trailing flush produces (flush
                # of round r runs during round r+2, same parity)
                pending.append((e_sb, nt, not big_on_act))
            for p in pending:
                flush(*p)

    nc.compile()
    return nc


def _host_prep(images, palettes, scale):
    """Per-core input arrays. images [16,256,256,3] f32, palettes [16,64,3].
    scale = 2/temperature; the softmax logit is scale*dot + bias_k."""
    import ml_dtypes

    bf = ml_dtypes.bfloat16
    imgs = np.ascontiguousarray(images, np.float32).reshape(B, P, C)
    pals = np.ascontiguousarray(palettes, np.float32)
    s1 = 1024.0 * LOG2E * scale
    in_maps = []
    ones2 = np.ones((2, QP), bf)
    for core in range(NCORES):
        ia, ib = imgs[2 * core], imgs[2 * core + 1]
        # per-quarter channel rows: [rgbA | rgbB] on the contraction dim
        x6 = np.empty((NQ, 6, QP), np.float32)
        x6[:, 0:3] = ia.reshape(NQ, QP, C).transpose(0, 2, 1)
        x6[:, 3:6] = ib.reshape(NQ, QP, C).transpose(0, 2, 1)
        xh = x6.astype(bf)
        xl = (x6 - xh.astype(np.float32)).astype(bf)
        xin = np.concatenate([xh, xl, xh], axis=1)           # [NQ, 18, QP]

        pa, pb = pals[2 * core], pals[2 * core + 1]
        p6 = np.zeros((6, 128), np.float64)   # block-diag [pA^T | pB^T]
        p6[0:3, 0:64] = pa.T
        p6[3:6, 64:128] = pb.T
        # per-partition logit bias (-|p|^2 * scale/2, i.e. -|p|^2/T)
        eb = np.empty((128,), np.float64)
        eb[0:64] = -0.5 * scale * (pa.astype(np.float64) ** 2).sum(-1)
        eb[64:128] = -0.5 * scale * (pb.astype(np.float64) ** 2).sum(-1)

        def make_palt(mult, bias):
            """[KR=20,128] = [qh|qh|ql|bias_h|bias_l] for q = mult*p6."""
            q = mult * p6
            qh = q.astype(bf)
            ql = (q - qh.astype(np.float64)).astype(bf)
            bh = bias.astype(bf)
            bl = (bias - bh.astype(np.float64)).astype(bf)
            rows = np.concatenate(
                [qh, qh, ql, bh[None, :], bl[None, :]], axis=0)
            full = np.zeros((128, 128), bf)
            for j in range(NQ):
                full[32 * j:32 * j + KR] = rows
            return full

        # ACT rounds: psum = scale*dot + eb;  e = exp(psum)
        palta = make_palt(scale, eb)
        # DVE rounds: psum = s1*dot + (1024*log2e*eb + 15360.5);
        # int16(psum) is the fp16 bit pattern of 2^(log2e*logit)
        paltd = make_palt(s1, 1024.0 * LOG2E * eb + 15360.5)

        palw = np.zeros((128, 8), np.float32)
        palw[0:64, 0:3] = pa
        palw[0:64, 3] = 1.0
        palw[64:128, 4:7] = pb
        palw[64:128, 7] = 1.0

        in_maps.append({"xin": xin, "ones": ones2, "palta": palta,
                        "paltd": paltd, "palw_hi": palw.astype(np.float16)})
    return in_maps


def _host_post(results):
    """results[core]["out"] [9, 128, 480] (numer rgb + denom, 2 images
    interleaved on col%8) -> [16, 256, 256, 3]."""
    # subtile s -> (round r, t): s = 12r + t; task i = 3r + t//4 ->
    # (strip j = i%4, chunk k = i//4), q = t%4;
    # pixel = j*QP + k*512 + q*128 + row; stored at block s//SPT col s%SPT
    s_arr = np.arange(NSUB)
    r_arr = np.minimum(s_arr // 12, (NTASKS + 2) // 3 - 1)
    t_arr = s_arr - 12 * r_arr
    i_arr = 3 * r_arr + t_arr // 4
    base = (i_arr % NQ) * QP + (i_arr // NQ) * RN + (t_arr % 4) * 128
    out = np.empty((B, P, C), np.float32)
    for core in range(NCORES):
        o = np.asarray(results[core]["out"], np.float32)  # [9,128,480]
        o = o.reshape(NBLK, 128, SPT, 2, 4)               # [b,row,u,img,ch]
        q = o[..., 0:3] / o[..., 3:4]
        dec = np.empty((IMGS_PER_CORE, P, C), np.float32)
        for s in range(NSUB):
            b, u = s // SPT, s % SPT
            dec[0, base[s]:base[s] + 128, :] = q[b, :, u, 0]
            dec[1, base[s]:base[s] + 128, :] = q[b, :, u, 1]
        out[2 * core] = dec[0]
        out[2 * core + 1] = dec[1]
    return out.reshape(B, H, W, C)


_CACHE = {}


def _get_nc(scale: float):
    key = (round(float(scale), 12), DVE_PAT, OUT_DT)
    if key not in _CACHE:
        _CACHE[key] = build_bass(scale)
    return _CACHE[key]


def kernel(images, palettes, temperature, _trace=False):
    scale = 2.0 / float(np.asarray(temperature))
    nc = _get_nc(scale)
    in_maps = _host_prep(images, palettes, scale)
    res = run_bass_kernel_spmd(nc, in_maps, core_ids=list(range(NCORES)),
                               trace=_trace)
    out = _host_post(res.results)
    if _trace:
        kernel.last_result = res
    return out


# revision 22
# speedup vs baseline: 1.2820x; 1.0292x over previous
"""Differentiable palette quantization on 8 Trainium2 NeuronCores.

Math: for each image b, pixel x, palette p_k (k=64):
    w = softmax_k(-|x - p_k|^2 / T);  out = sum_k w_k p_k
Softmax is invariant to the per-pixel |x|^2 term, so the logit reduces to
    scale*dot(x, p_k) + bias_k,  scale = 2/T, bias_k = -|p_k|^2/T.

Sharding: pure data parallel, 2 images per core, stacked on partitions
(64+64 palette entries) sharing the pixel stream via a block-diagonal
stationary matrix.  The dot matmul runs in bf16 with an exact hi/lo
fixup folded into the contraction dim.

v3 design (vs the 82us ACT-bound baseline):
 1. The exp is SPLIT between ACT and DVE.  Even rounds run the real exp
    on ACT (fp16 out).  Odd rounds run on DVE as an exp2 bit-trick: the
    int16 value round(1024*log2(e)*logit + 15360) IS the fp16 bit
    pattern of 2^y ~ e^logit (Schraudolph).  Softmax cancels the
    systematic part; measured end-to-end error ~7e-3 vs the 2e-2 gate.
 2. Both the scale and the bias of each engine's exp input are folded
    into the mm1 contraction: two per-engine stationary palettes
    (scale*p for ACT, 1024*log2e*scale*p for DVE) carry two extra hi/lo
    bias rows against constant-1.0 pixel rows.  ACT runs exp with
    scale=1/bias=0; DVE is a single convert-to-int16 tensor_scalar.
 3. The PE queue issues round r+1's three mm1 matmuls BEFORE round r's
    twelve weighted-sum matmuls, so the ACT exp of round r and the DVE
    exp of round r+1 genuinely overlap (the in-order PE queue otherwise
    serializes the two engines).
 4. No divide epilogue: psum2 blocks (numerators + softmax denominators)
    are copied PSUM->SBUF as fp16 by whichever of ACT/DVE the round
    parity frees, DMA'd raw to DRAM, and the host divides.
 5. DMA queues: input pixel chunks on sync HWDGE, palettes/constants on
    gpsimd SWDGE, output blocks on gpsimd with the last two on sync -
    the Scalar and Vector queues carry no DMA.
"""

import os
import sys

for _p in ("/opt/trn_rl_repo", os.path.expanduser("~/.axon_site/_ro/trn_rl_repo")):
    if os.path.isdir(_p) and _p not in sys.path:
        sys.path.insert(0, _p)

import numpy as np

import concourse.bass as bass
import concourse.tile as tile
from concourse import bacc, mybir
from concourse.bass_utils import run_bass_kernel_spmd

# problem constants (hardcoded per contract)
B, H, W, C, K = 16, 256, 256, 3, 64
NCORES = 8
IMGS_PER_CORE = B // NCORES            # 2
P = H * W                              # 65536 pixel-pairs per core
NQ = 4                                 # PE row-tile quarters
QP = P // NQ                           # 16384 pixels per quarter
RN = 512                               # pixels per strip-task
ROUNDS = QP // RN                      # 32 chunks per strip
NTASKS = NQ * ROUNDS                   # 128 matmul tasks
SPT = 64                               # weighted-sum subtiles per psum2 block
NSUB = NTASKS * 4                      # 512 subtiles of 128 pixels
NBLK = (NSUB + SPT - 1) // SPT         # 8 output blocks (1 full PSUM bank)
KR = 20                                # contraction rows: 3x6 hi/lo + 2 bias

LOG2E = float(np.log2(np.e))

# tuning knobs (env-overridable for experiments)
DVE_PAT = os.environ.get("PALQ_DVE_PAT", "AD")  # cycle of A/D per round
OUT_DT = os.environ.get("PALQ_OUT_DT", "float16")  # psum2 copy-out dtype


def build_bass(scale: float):
    nc = bacc.Bacc("TRN2", target_bir_lowering=False, debug=False)
    f32 = mybir.dt.float32
    f16 = mybir.dt.float16
    bf16 = mybir.dt.bfloat16
    out_dt = getattr(mybir.dt, OUT_DT)
    i16 = mybir.dt.int16

    xin = nc.dram_tensor("xin", [NQ, 18, QP], bf16, kind="ExternalInput")
    ones = nc.dram_tensor("ones", [2, QP], bf16, kind="ExternalInput")
    palta = nc.dram_tensor("palta", [128, 128], bf16, kind="ExternalInput")
    paltd = nc.dram_tensor("paltd", [128, 128], bf16, kind="ExternalInput")
    palw_hi = nc.dram_tensor("palw_hi", [128, 8], f16, kind="ExternalInput")
    out = nc.dram_tensor("out", [NBLK, 128, 8 * SPT], out_dt,
                         kind="ExternalOutput")

    with tile.TileContext(nc) as tc:
        import contextlib
        with contextlib.ExitStack() as ctx:
            singles = ctx.enter_context(tc.tile_pool(name="singles", bufs=1))
            epool = ctx.enter_context(tc.tile_pool(name="epool", bufs=4))
            # psum1 = two pools per round: a 2-bank [1024] tile (tasks 0,1)
            # and a 1-bank [512] tile (task 2), each double buffered.  The
            # round's exp runs as ONE op per engine (1024 on one, 512 on
            # the other, pair-alternating), so the exp makespan stays under
            # the PE's ~1850ns pair budget.  4+2+2 = 8 PSUM banks.
            psA = ctx.enter_context(tc.tile_pool(name="psA", bufs=2, space="PSUM"))
            psB = ctx.enter_context(tc.tile_pool(name="psB", bufs=2, space="PSUM"))
            ps2 = ctx.enter_context(tc.tile_pool(name="ps2", bufs=2, space="PSUM"))
            opool = ctx.enter_context(tc.tile_pool(name="opool", bufs=3))

            # resident input pixels: quarter j on partitions [32j, 32j+20);
            # rows 18,19 are the constant-1.0 bias rows
            xsb = singles.tile([128, QP], bf16)

            # round 0 is all-ACT on strips 0,1,2: palta + those chunks go
            # FIRST on sync.  paltd + strip-3 ride the gpsimd queue.
            palta_sb = singles.tile([128, 128], bf16)
            nc.sync.dma_start(out=palta_sb, in_=palta.ap())
            for j in range(3):
                nc.sync.dma_start(out=xsb[32 * j:32 * j + 18, 0:512],
                                  in_=xin.ap()[j, :, 0:512])
            paltd_sb = singles.tile([128, 128], bf16)
            nc.gpsimd.dma_start(out=paltd_sb, in_=paltd.ap())
            nc.gpsimd.dma_start(out=xsb[96:96 + 18, 0:512],
                                in_=xin.ap()[3, :, 0:512])
            for j in range(NQ):
                nc.gpsimd.dma_start(out=xsb[32 * j + 18:32 * j + 20, :],
                                    in_=ones.ap())
            palw_sb = singles.tile([128, 8], f16)
            nc.gpsimd.dma_start(out=palw_sb, in_=palw_hi.ap())

            # pre-warm the ACT exp table while input DMAs stream
            warm = singles.tile([1, 1], f32)
            nc.scalar.activation(out=warm,
                                 in_=nc.const_aps.scalar_like(0.0, warm),
                                 func=mybir.ActivationFunctionType.Exp)

            # remaining pixel chunks on sync; chunk-major issue order with
            # small first chunks so round 0's columns land early
            bounds = [0, 512, 1536, 2560, 4096, 6144, 8192, 10240, 12288,
                      14336, QP]
            for h in range(len(bounds) - 1):
                sl = slice(bounds[h], bounds[h + 1])
                for j in range(NQ):
                    if h == 0:
                        continue  # issued above
                    # strips 0,1 feed from sync; strips 2,3 from gpsimd so
                    # neither queue's descriptor-gen rate starves the PE
                    eng = nc.sync if j < 2 else nc.gpsimd
                    eng.dma_start(out=xsb[32 * j:32 * j + 18, sl],
                                  in_=xin.ap()[j, :, sl])

            # 128 matmul tasks i -> (strip j = i%4, chunk k = i//4), three
            # per 1536-col round (strips always distinct mod 4).  psum1 =
            # 2x3 banks, psum2 = 2x1 banks: exactly 8 PSUM banks.
            # Round r+1's mm1s are issued BEFORE round r's mm2s so the two
            # exp engines overlap; mm2s trail one round behind.
            NRND = (NTASKS + 2) // 3               # 43 (last has 2 tasks)
            pat = DVE_PAT

            state = {"psum2": None, "s": 0, "flushed": 0}

            def flush(e_sb, nt, use_dve):
                """12 weighted-sum matmuls for a finished e tile + block
                copies when psum2 fills.  The engine NOT running exps this
                round does the PSUM->SBUF copy."""
                s = state["s"]
                for t in range(4 * nt):
                    if state["psum2"] is None:
                        state["psum2"] = ps2.tile([128, 8 * SPT], f32,
                                                  name="psum2")
                    psum2 = state["psum2"]
                    u = s % SPT
                    nc.tensor.matmul(
                        out=psum2[:, 8 * u:8 * u + 8],
                        lhsT=e_sb[:, 128 * t:128 * (t + 1)],
                        rhs=palw_sb,
                        start=True, stop=True,
                    )
                    s += 1
                    if s % SPT == 0 or s == NSUB or s == NSUB - 16:
                        # ship psum2 subtiles [flushed, s): the last block
                        # goes in two halves so its copy+DMA overlap the
                        # final rounds' compute
                        b = (s - 1) // SPT
                        v0 = state["flushed"] - SPT * b
                        v1 = (s - 1) % SPT + 1
                        state["flushed"] = s
                        nu = v1 - v0
                        ob = opool.tile([128, 8 * nu], out_dt, name="ob")
                        if use_dve:
                            nc.scalar.activation(
                                out=ob, in_=psum2[:, 8 * v0:8 * v1],
                                func=mybir.ActivationFunctionType.Copy)
                        else:
                            nc.vector.tensor_scalar_add(
                                out=ob, in0=psum2[:, 8 * v0:8 * v1],
                                scalar1=0.0)
                        oeng = nc.gpsimd if b < NBLK - 2 else nc.sync
                        oeng.dma_start(out=out.ap()[b, :, 8 * v0:8 * v1],
                                       in_=ob)
                        if s % SPT == 0 or s == NSUB:
                            state["psum2"] = None
                state["s"] = s

            # mm2s trail TWO rounds behind their exp: the in-order PE queue
            # then never stalls (each mm2 batch consumes an e tile finished
            # two rounds ago) and the two exp engines overlap fully
            # Engine split per round: tasks 0,1 (the 1024-col psA tile) on
            # one engine, task 2 (psB, 512 cols) on the other; which engine
            # gets the big slice alternates per round, balancing ACT's
            # 0.83ns/col against DVE's 1.04ns/col over round pairs.
            pending = []
            for r in range(NRND):
                tasks = [3 * r + m for m in range(3) if 3 * r + m < NTASKS]
                nt = len(tasks)
                big_on_act = r % 2 == 0
                psum1a = psA.tile([128, 2 * RN], f32, name="psum1a")
                psum1b = psB.tile([128, RN], f32, name="psum1b") \
                    if nt == 3 else None
                for m, i in enumerate(tasks):
                    j, k = i % NQ, i // NQ
                    psl = slice(32 * j, 32 * j + KR)
                    on_act = big_on_act if m < 2 else not big_on_act
                    nc.tensor.matmul(
                        out=psum1a[:, RN * m:RN * (m + 1)] if m < 2
                        else psum1b[:, 0:RN],
                        lhsT=(palta_sb if on_act else paltd_sb)[psl, :],
                        rhs=xsb[psl, RN * k:RN * (k + 1)],
                        start=True, stop=True,
                        tile_position=(32 * j, 0),
                    )
                if len(pending) == 2:
                    flush(*pending.pop(0))
                e_sb = epool.tile([128, 3 * RN], f16)
                segs = [(psum1a, 0, 2 * RN)]
                if nt == 3:
                    segs.append((psum1b, 2 * RN, 3 * RN))
                for si, (src, c0, c1) in enumerate(segs):
                    on_act = big_on_act if si == 0 else not big_on_act
                    if on_act:
                        nc.scalar.activation(
                            out=e_sb[:, c0:c1], in_=src[:, 0:c1 - c0],
                            func=mybir.ActivationFunctionType.Exp,
                        )
                    else:
                        nc.vector.tensor_scalar_add(
                            out=e_sb[:, c0:c1].bitcast(i16),
                            in0=src[:, 0:c1 - c0],
                            scalar1=0.0,
                        )
                # the engine holding only the 512-col slice this round owns
                # any psum2 block copy the trailing flush produces (flush
                # of round r runs during round r+2, same parity)
                pending.append((e_sb, nt, not big_on_act))
            for p in pending:
                flush(*p)

    nc.compile()
    return nc


def _host_prep(images, palettes, scale):
    """Per-core input arrays. images [16,256,256,3] f32, palettes [16,64,3].
    scale = 2/temperature; the softmax logit is scale*dot + bias_k."""
    import ml_dtypes

    bf = ml_dtypes.bfloat16
    imgs = np.ascontiguousarray(images, np.float32).reshape(B, P, C)
    pals = np.ascontiguousarray(palettes, np.float32)
    s1 = 1024.0 * LOG2E * scale
    in_maps = []
    ones2 = np.ones((2, QP), bf)
    for core in range(NCORES):
        ia, ib = imgs[2 * core], imgs[2 * core + 1]
        # per-quarter channel rows: [rgbA | rgbB] on the contraction dim
        x6 = np.empty((NQ, 6, QP), np.float32)
        x6[:, 0:3] = ia.reshape(NQ, QP, C).transpose(0, 2, 1)
        x6[:, 3:6] = ib.reshape(NQ, QP, C).transpose(0, 2, 1)
        xh = x6.astype(bf)
        xl = (x6 - xh.astype(np.float32)).astype(bf)
        xin = np.concatenate([xh, xl, xh], axis=1)           # [NQ, 18, QP]

        pa, pb = pals[2 * core], pals[2 * core + 1]
        p6 = np.zeros((6, 128), np.float64)   # block-diag [pA^T | pB^T]
        p6[0:3, 0:64] = pa.T
        p6[3:6, 64:128] = pb.T
        # per-partition logit bias (-|p|^2 * scale/2, i.e. -|p|^2/T)
        eb = np.empty((128,), np.float64)
        eb[0:64] = -0.5 * scale * (pa.astype(np.float64) ** 2).sum(-1)
        eb[64:128] = -0.5 * scale * (pb.astype(np.float64) ** 2).sum(-1)

        def make_palt(mult, bias):
            """[KR=20,128] = [qh|qh|ql|bias_h|bias_l] for q = mult*p6."""
            q = mult * p6
            qh = q.astype(bf)
            ql = (q - qh.astype(np.float64)).astype(bf)
            bh = bias.astype(bf)
            bl = (bias - bh.astype(np.float64)).astype(bf)
            rows = np.concatenate(
                [qh, qh, ql, bh[None, :], bl[None, :]], axis=0)
            full = np.zeros((128, 128), bf)
            for j in range(NQ):
                full[32 * j:32 * j + KR] = rows
            return full

        # ACT rounds: psum = scale*dot + eb;  e = exp(psum)
        palta = make_palt(scale, eb)
        # DVE rounds: psum = s1*dot + (1024*log2e*eb + 15360.5);
        # int16(psum) is the fp16 bit pattern of 2^(log2e*logit)
        paltd = make_palt(s1, 1024.0 * LOG2E * eb + 15360.5)

        palw = np.zeros((128, 8), np.float32)
        palw[0:64, 0:3] = pa
        palw[0:64, 3] = 1.0
        palw[64:128, 4:7] = pb
        palw[64:128, 7] = 1.0

        in_maps.append({"xin": xin, "ones": ones2, "palta": palta,
                        "paltd": paltd, "palw_hi": palw.astype(np.float16)})
    return in_maps


def _host_post(results):
    """results[core]["out"] [9, 128, 480] (numer rgb + denom, 2 images
    interleaved on col%8) -> [16, 256, 256, 3]."""
    # subtile s -> (round r, t): s = 12r + t; task i = 3r + t//4 ->
    # (strip j = i%4, chunk k = i//4), q = t%4;
    # pixel = j*QP + k*512 + q*128 + row; stored at block s//SPT col s%SPT
    s_arr = np.arange(NSUB)
    r_arr = np.minimum(s_arr // 12, (NTASKS + 2) // 3 - 1)
    t_arr = s_arr - 12 * r_arr
    i_arr = 3 * r_arr + t_arr // 4
    base = (i_arr % NQ) * QP + (i_arr // NQ) * RN + (t_arr % 4) * 128
    out = np.empty((B, P, C), np.float32)
    for core in range(NCORES):
        o = np.asarray(results[core]["out"], np.float32)  # [9,128,480]
        o = o.reshape(NBLK, 128, SPT, 2, 4)               # [b,row,u,img,ch]
        q = o[..., 0:3] / o[..., 3:4]
        dec = np.empty((IMGS_PER_CORE, P, C), np.float32)
        for s in range(NSUB):
            b, u = s // SPT, s % SPT
            dec[0, base[s]:base[s] + 128, :] = q[b, :, u, 0]
            dec[1, base[s]:base[s] + 128, :] = q[b, :, u, 1]
        out[2 * core] = dec[0]
        out[2 * core + 1] = dec[1]
    return out.reshape(B, H, W, C)


_CACHE = {}


def _get_nc(scale: float):
    key = (round(float(scale), 12), DVE_PAT, OUT_DT)
    if key not in _CACHE:
        _CACHE[key] = build_bass(scale)
    return _CACHE[key]


def kernel(images, palettes, temperature, _trace=False):
    scale = 2.0 / float(np.asarray(temperature))
    nc = _get_nc(scale)
    in_maps = _host_prep(images, palettes, scale)
    res = run_bass_kernel_spmd(nc, in_maps, core_ids=list(range(NCORES)),
                               trace=_trace)
    out = _host_post(res.results)
    if _trace:
        kernel.last_result = res
    return out
